# revision 22
# baseline (speedup 1.0000x reference)
"""GCN (single GCNConv + ELU) forward on 8 Trainium2 NeuronCores.

out = ELU( D^-1/2 (A + I) D^-1/2 (x @ W) + b )

V2 strategy (1D dst-partition, slot-major tier-1 + one-hot tier-2):
  - Nodes (dst rows) sharded across 8 cores; edges partitioned by dst.
  - Host pre-scales x by dinv[node] so phase A is a pure matmul:
    h' = (x * dinv) @ W, per-core shard, then AllGather -> hp_full in DRAM.
  - The per-edge norm coef dinv[src]*dinv[dst] factors: dinv[src] rides h',
    dinv[dst] is applied once per destination row after aggregation.
  - Tier-1: every dst gets S[k] gather slots per src-bucket (int16 dma_gather
    indices, 4 src buckets of 32768 padded rows). Gathered slot-major:
    G[p=dst, slot, 64]. Segment-sum = ONE strided tensor_reduce per
    (block-group, bucket). Unused slots point at known zero rows of hp_full.
  - Tier-2: excess edges (degree tail) go through a selection-matrix matmul:
    S_T[e, j] = (j == dstloc[e]) built by one tensor_scalar per 128-edge
    piece, psum += S_T.T @ G on the PE per 128-dst block.
  - Epilogue: z = (t1 + t2 + h'_own) * dinv_dst (+ b); out = ELU(z).
  - dma_gather calls are <=1024 rows (single_packet crashes above that),
    round-robined over 4 SWDGE queues (4x faster descriptor generation).

Self-contained: no imports from the problem directory.
"""

import math
import os
import sys

import ml_dtypes
import numpy as np

sys.path.insert(0, "/opt/trn_rl_repo")

import concourse.bacc as bacc  # noqa: E402
import concourse.bass as bass  # noqa: E402
import concourse.library_config as library_config  # noqa: E402
import concourse.mybir as mybir  # noqa: E402
import concourse.tile as tile  # noqa: E402
from concourse import bass_utils  # noqa: E402

F32 = mybir.dt.float32
BF16 = mybir.dt.bfloat16
I16 = mybir.dt.int16
AF = mybir.ActivationFunctionType
OP = mybir.AluOpType
AX = mybir.AxisListType

P = 128
BW = 32768          # src bucket width (int16 gather index limit)
CALL_ROWS = 1024    # max rows per dma_gather call (single_packet limit)
NQ = 4              # SWDGE queues


class Cfg:
    def __init__(self, N, E, F, H, C, GB=5, S=(4, 4, 4, 4), NCHK=4):
        self.N, self.E, self.F, self.H, self.C = N, E, F, H, C
        assert N % C == 0
        self.NPC = N // C
        self.NBLK = math.ceil(self.NPC / P)
        self.NPCPAD = self.NBLK * P
        # chunk-major hp layout: shard split into NCHK chunks; each chunk
        # gets one extra all-zero pad block. Bucket k == chunk k of all
        # cores, so AllGather can be chunked and pipelined with gathers.
        self.NCHK = NCHK
        bpc = math.ceil(self.NBLK / NCHK)           # real blocks per chunk
        self.RBCH = [bpc] * (NCHK - 1) + [self.NBLK - bpc * (NCHK - 1)]
        self.CHB = [rb + 1 for rb in self.RBCH]     # +1 zero block
        cb = [0]
        for w in self.CHB:
            cb.append(cb[-1] + w)
        self.CB = cb                                 # chunk block offsets
        self.SHROWS = cb[-1] * P
        self.NTOTPAD = C * self.SHROWS
        self.NBKT = NCHK
        self.KBASE = [C * cb[k] * P for k in range(NCHK)]
        self.KWID = [C * self.CHB[k] * P for k in range(NCHK)]
        assert all(w <= 32767 for w in self.KWID), self.KWID
        self.GB = GB
        self.NGRP = math.ceil(self.NBLK / GB)
        self.S = list(S)[:self.NBKT]
        if len(self.S) < self.NBKT:
            self.S += [1] * (self.NBKT - len(self.S))

    def src_chunk(self, b):
        """chunk index of a real shard block b."""
        return min(b // (self.RBCH[0]), self.NCHK - 1)


def _group_blocks(cfg, g):
    return range(g * cfg.GB, min((g + 1) * cfg.GB, cfg.NBLK))


def _zero_rows(cfg):
    """Bucket-relative row of core 0's all-zero pad block in each bucket."""
    return {k: cfg.RBCH[k] * P for k in range(cfg.NBKT)}


def _preprocess(cfg, x, W, b, edge_index):
    N, C, NPC, NBLK, NBKT = cfg.N, cfg.C, cfg.NPC, cfg.NBLK, cfg.NBKT
    S = cfg.S
    src = np.asarray(edge_index[0], dtype=np.int64)
    dst = np.asarray(edge_index[1], dtype=np.int64)

    deg = np.bincount(dst, minlength=N).astype(np.float64) + 1.0
    dinv = (1.0 / np.sqrt(deg)).astype(np.float32)
    # chunk-major bucket-relative row of each src node
    sc = src // NPC
    sl = src % NPC
    sb = sl >> 7
    bpc = cfg.RBCH[0]
    sj = np.minimum(sb // bpc, cfg.NCHK - 1)
    chb = np.array(cfg.CHB, dtype=np.int64)
    rel_all = (sc * chb[sj] + (sb - sj * bpc)) * P + (sl & 127)
    bk_all = sj
    zrows = _zero_rows(cfg)
    has_bias = bool(np.any(np.asarray(b) != 0))

    # ---------- per-core structure ----------
    slot_base = np.concatenate([[0], np.cumsum(S)]).astype(np.int64)
    TSLOT = int(slot_base[-1])          # tier-1 slots per dst

    cores = []
    for c in range(C):
        sel = (dst // NPC) == c
        es = rel_all[sel]                   # bucket-relative rows
        ed = (dst[sel] - c * NPC).astype(np.int64)
        bk = bk_all[sel]
        # rank within (dst, bucket)
        order = np.lexsort((es, bk, ed))
        es, ed, bk = es[order], ed[order], bk[order]
        gkey = ed * NBKT + bk
        newseg = np.empty(len(gkey), dtype=bool)
        newseg[0:1] = True
        newseg[1:] = gkey[1:] != gkey[:-1]
        segstart = np.maximum.accumulate(np.where(newseg, np.arange(len(gkey)), 0))
        rank = np.arange(len(gkey)) - segstart
        capk = np.array(S, dtype=np.int64)[bk]
        t1 = rank < capk
        cores.append(dict(es=es, ed=ed, bk=bk, rank=rank, t1=t1))

    # tier-2 tile counts per (group, bucket) must be shared across cores
    t2cnt = np.zeros((C, cfg.NGRP, NBKT), dtype=np.int64)
    for c in range(C):
        d = cores[c]
        m = ~d["t1"]
        gi = d["ed"][m] >> 7
        grp = gi // cfg.GB
        np.add.at(t2cnt[c], (grp, d["bk"][m]), 1)
    T2 = np.ceil(t2cnt / P).astype(np.int64).max(axis=0)    # [NGRP, NBKT] tiles

    # ---------- global row-stream layout (shared) ----------
    # per group g: for k: [tier1: GBcur*S_k*128 rows][tier2: T2[g,k]*128 rows]
    regions = []        # (g, k, kind, row0, nrows)
    row = 0
    for g in range(cfg.NGRP):
        gb = len(_group_blocks(cfg, g))
        for k in range(NBKT):
            n1 = gb * S[k] * P
            regions.append((g, k, 1, row, n1))
            row += n1
            n2 = int(T2[g, k]) * P
            if n2:
                regions.append((g, k, 2, row, n2))
                row += n2
    TOTROWS = row
    region_map = {(g, k, kind): (r0, nr) for (g, k, kind, r0, nr) in regions}

    # gather calls: chunks of <=CALL_ROWS within each region
    calls = []
    for (g, k, kind, r0, nr) in regions:
        off = 0
        while off < nr:
            n = min(CALL_ROWS, nr - off)
            calls.append(dict(g=g, k=k, row0=r0 + off, rows=n,
                              q=len(calls) % NQ))
            off += n

    # tier-2 pieces (shared static structure): for (g,k) tile j, pieces are
    # computed per-core but piece boundaries differ per core! -> make the
    # PIECE LIST static: one piece per (tile, block) for every block in the
    # group: a piece's dstloc column zeroes out non-member rows, so a static
    # piece list valid for all cores is: for each (g,k) tile, all blocks of
    # group g. That would be GB pieces per tile (too many); instead take the
    # union over cores of blocks actually touched by that tile.
    piece_map = {}       # (g,k,j) -> sorted list of blocks
    for c in range(C):
        d = cores[c]
        m = ~d["t1"]
        es2, ed2, bk2 = d["es"][m], d["ed"][m], d["bk"][m]
        o2 = np.lexsort((ed2, bk2, (ed2 >> 7) // cfg.GB))
        es2, ed2, bk2 = es2[o2], ed2[o2], bk2[o2]
        d["t2_sorted"] = (es2, ed2, bk2)
        # positions within (g,k) region
        grp2 = (ed2 >> 7) // cfg.GB
        for (g, k, kind, r0, nr) in regions:
            if kind != 2:
                continue
            mm = (grp2 == g) & (bk2 == k)
            edm = ed2[mm]
            if len(edm) == 0:
                continue
            for j in range(int(T2[g, k])):
                seg = edm[j * P:(j + 1) * P]
                if len(seg) == 0:
                    break
                for blk in np.unique(seg >> 7):
                    piece_map.setdefault((g, k, j), set()).add(int(blk))
    # order pieces by (g, blk, k, j) so each block's pieces are a
    # contiguous ci-run (enables one batched is_equal per block)
    flat = []
    for (g, k, j), blks in sorted(piece_map.items()):
        for blk in sorted(blks):
            flat.append((g, blk, k, j))
    flat.sort()
    piece_list = [(g, k, j, blk, ci)
                  for ci, (g, blk, k, j) in enumerate(flat)]
    NPIECE = len(piece_list)

    # ---------- per-core arrays ----------
    in_maps = []
    xs = np.asarray(x, dtype=np.float32)
    for c in range(C):
        d = cores[c]
        idx_stream = np.zeros(TOTROWS, dtype=np.int16)
        # default: every row points at the zero row of its region's bucket
        for (g, k, kind, r0, nr) in regions:
            idx_stream[r0:r0 + nr] = zrows[k]

        # tier-1 fill
        t1m = d["t1"]
        es1, ed1, bk1, rk1 = (d["es"][t1m], d["ed"][t1m], d["bk"][t1m],
                              d["rank"][t1m])
        blk1 = ed1 >> 7
        g1 = blk1 // cfg.GB
        gb_base = (g1 * cfg.GB)
        b_in_g = blk1 - gb_base
        slot = slot_base[bk1] + rk1          # 0..TSLOT-1 (bucket-sectioned)
        slot_in_k = rk1
        # row = region(g,k,1).row0 + (b_in_g * S_k + slot_in_k)*128 + p
        r0s = np.array([[region_map[(g, k, 1)][0] for k in range(NBKT)]
                        for g in range(cfg.NGRP)], dtype=np.int64)
        Sarr = np.array(S, dtype=np.int64)
        rows1 = (r0s[g1, bk1] + (b_in_g * Sarr[bk1] + slot_in_k) * P
                 + (ed1 & 127))
        idx_stream[rows1] = es1.astype(np.int16)

        # tier-2 fill + dstloc per piece
        dstloc2 = np.full((P, max(NPIECE, 1)), -1.0, dtype=np.float32)
        es2, ed2, bk2 = d["t2_sorted"]
        grp2 = (ed2 >> 7) // cfg.GB
        pos = 0
        pc_index = {(g, k, j, blk): ci for (g, k, j, blk, ci) in piece_list}
        for g in range(cfg.NGRP):
            for k in range(NBKT):
                mm = (grp2 == g) & (bk2 == k)
                nseg = int(mm.sum())
                if (g, k, 2) not in region_map:
                    assert nseg == 0
                    continue
                r0, nr = region_map[(g, k, 2)]
                esg, edg = es2[mm], ed2[mm]
                assert nseg <= nr
                idx_stream[r0:r0 + nseg] = esg.astype(np.int16)
                for j in range(int(T2[g, k])):
                    seg_ed = edg[j * P:(j + 1) * P]
                    if len(seg_ed) == 0:
                        break
                    for blk in np.unique(seg_ed >> 7):
                        ci = pc_index[(g, k, j, int(blk))]
                        rows_in_tile = np.nonzero((seg_ed >> 7) == blk)[0]
                        dstloc2[rows_in_tile, ci] = (seg_ed[rows_in_tile]
                                                     & 127).astype(np.float32)
        # wrapped idx per call, replicated to 128 partitions
        idx16 = np.zeros((16, TOTROWS // 16), dtype=np.int16)
        for cl in calls:
            r0, nr = cl["row0"], cl["rows"]
            idx16[:, r0 // 16:(r0 + nr) // 16] = \
                idx_stream[r0:r0 + nr].reshape(-1, 16).T
        idx_dram = np.tile(idx16, (8, 1))

        xc = np.zeros((cfg.NPCPAD, cfg.F), dtype=np.float32)
        xc[:NPC] = xs[c * NPC:(c + 1) * NPC] * dinv[c * NPC:(c + 1) * NPC, None]
        dv = np.zeros(cfg.NPCPAD, dtype=np.float32)
        dv[:NPC] = dinv[c * NPC:(c + 1) * NPC]
        m = {
            "iotaf": np.broadcast_to(
                np.arange(P, dtype=np.float32), (P, P)).copy(),
            "xT": np.ascontiguousarray(xc.T).astype(ml_dtypes.bfloat16),
            "Wm": np.ascontiguousarray(W, dtype=np.float32).astype(
                ml_dtypes.bfloat16),
            "dinv_own": np.ascontiguousarray(dv.reshape(NBLK, P).T),
            "dstloc2": np.ascontiguousarray(dstloc2),
            "idx": idx_dram,
        }
        if has_bias:
            m["bvec"] = np.ascontiguousarray(
                np.broadcast_to(np.asarray(b, np.float32), (P, cfg.H)))
        in_maps.append(m)

    meta = dict(T2=T2, regions=regions, region_map=region_map, calls=calls,
                piece_list=piece_list, NPIECE=NPIECE, TOTROWS=TOTROWS,
                has_bias=has_bias, TSLOT=TSLOT)
    return in_maps, meta


def _build_program(cfg, meta):
    C, H, F, NBLK, NBKT = cfg.C, cfg.H, cfg.F, cfg.NBLK, cfg.NBKT
    S = cfg.S
    T2 = meta["T2"]
    region_map = meta["region_map"]
    calls = meta["calls"]
    piece_list = meta["piece_list"]
    NPIECE = meta["NPIECE"]
    TOTROWS = meta["TOTROWS"]
    has_bias = meta["has_bias"]
    TSLOT = meta["TSLOT"]

    nc = bacc.Bacc(num_devices=C, num_swdge_queues=NQ)
    xT = nc.dram_tensor("xT", [F, cfg.NPCPAD], BF16, kind="ExternalInput")
    Wm = nc.dram_tensor("Wm", [F, H], BF16, kind="ExternalInput")
    dinv_d = nc.dram_tensor("dinv_own", [P, NBLK], F32, kind="ExternalInput")
    dl2_d = nc.dram_tensor("dstloc2", [P, max(NPIECE, 1)], F32,
                           kind="ExternalInput")
    idx_d = nc.dram_tensor("idx", [P, TOTROWS // 16], I16, kind="ExternalInput")
    iota_d = nc.dram_tensor("iotaf", [P, P], F32, kind="ExternalInput")
    if has_bias:
        bvec_d = nc.dram_tensor("bvec", [P, H], F32, kind="ExternalInput")
    out_d = nc.dram_tensor("out", [cfg.NPCPAD, H], F32, kind="ExternalOutput")

    calls_by_g = {}
    for cl in calls:
        calls_by_g.setdefault(cl["g"], []).append(cl)
    pieces_by_g = {}
    for (g, k, j, blk, ci) in piece_list:
        pieces_by_g.setdefault(g, []).append((k, j, blk, ci))

    with tile.TileContext(nc, num_cores=C) as tc:
        with (
            tc.tile_pool(name="const", bufs=1) as cpool,
            tc.tile_pool(name="dram", bufs=1, space="DRAM") as dram,
            tc.tile_pool(name="xa", bufs=3) as xa_pool,
            tc.tile_pool(name="psA", bufs=1, space="PSUM") as psA,
            tc.tile_pool(name="psB", bufs=4, space="PSUM") as psB,
            tc.tile_pool(name="st", bufs=2) as st_pool,
            tc.tile_pool(name="g1", bufs=3) as g1_pool,
            tc.tile_pool(name="part", bufs=3) as part_pool,
            tc.tile_pool(name="idxp", bufs=3) as idx_pool,
            tc.tile_pool(name="ep", bufs=2) as e_pool,
        ):
            nc.gpsimd.load_library(library_config.mlp)
            W_sb = cpool.tile([F, H], BF16)
            nc.sync.dma_start(out=W_sb[:], in_=Wm[:, :])
            dinv_sb = cpool.tile([P, NBLK], F32)
            nc.sync.dma_start(out=dinv_sb[:], in_=dinv_d[:, :])
            dl2_sb = cpool.tile([P, max(NPIECE, 1)], F32)
            nc.sync.dma_start(out=dl2_sb[:], in_=dl2_d[:, :])
            iota_f = cpool.tile([P, P], F32)
            nc.sync.dma_start(out=iota_f[:], in_=iota_d[:, :])
            if has_bias:
                b_sb = cpool.tile([P, H], F32)
                nc.sync.dma_start(out=b_sb[:], in_=bvec_d[:, :])
            hp_own = cpool.tile([P, NBLK * H], F32)
            zbuf = cpool.tile([P, NBLK * H], F32)

            # ---- phase A: h' shard, chunk-major, pipelined AllGathers ----
            # one DRAM tensor per chunk so Tile dependencies let bucket-k
            # gathers start right after collective k (not the whole gather)
            hp_shards = [dram.tile([cfg.CHB[k] * P, H], F32,
                                   name=f"hpsh{k}") for k in range(NBKT)]
            hp_fulls = [dram.tile([cfg.KWID[k], H], F32, addr_space="Shared",
                                  name=f"hpf{k}") for k in range(NBKT)]
            zt = xa_pool.tile([P, H], F32, tag="zt")
            nc.vector.memset(zt[:], 0.0)
            bigp = psA.tile([P, 2048], F32)
            bpc = cfg.RBCH[0]
            XB = 4                           # x blocks per load
            for k in range(NBKT):
                tlo = k * bpc
                nb = cfg.RBCH[k]             # real blocks in this chunk
                for i0 in range(0, nb, XB):
                    nx = min(XB, nb - i0)
                    xt = xa_pool.tile([P, nx * P], BF16, tag="xt",
                                      name=f"xt{k}_{i0}")
                    nc.sync.dma_start(
                        out=xt[:],
                        in_=xT[:, (tlo + i0) * P:(tlo + i0 + nx) * P])
                    for i in range(nx):
                        j = i0 + i
                        nc.tensor.matmul(out=bigp[:, j * H:(j + 1) * H],
                                         lhsT=xt[:, i * P:(i + 1) * P],
                                         rhs=W_sb[:], start=True, stop=True)
                nc.scalar.activation(
                    out=hp_own[:, tlo * H:(tlo + nb) * H],
                    in_=bigp[:, :nb * H], func=AF.Copy)
                # one batched store: SBUF [p, (lb, h)] -> DRAM [(lb, p), h]
                st_in = bass.AP(hp_own.tensor, hp_own[:].offset + tlo * H,
                                [hp_own[:].ap[0], [H, nb], [1, H]])
                st_out = bass.AP(hp_shards[k].tensor,
                                 hp_shards[k][:].offset,
                                 [[H, P], [P * H, nb], [1, H]])
                nc.sync.dma_start(out=st_out, in_=st_in)
                zb = cfg.RBCH[k]             # zero block position
                nc.sync.dma_start(out=hp_shards[k][zb * P:(zb + 1) * P, :],
                                  in_=zt[:])
                nc.gpsimd.collective_compute(
                    "AllGather", OP.bypass,
                    replica_groups=[list(range(C))],
                    ins=[hp_shards[k][:, :]], outs=[hp_fulls[k][:, :]])

            # ---- phase B ----
            def _grp_meta(g):
                r0_g = region_map[(g, 0, 1)][0]
                rows_g = 0
                for k in range(NBKT):
                    rows_g += region_map[(g, k, 1)][1]
                    if (g, k, 2) in region_map:
                        rows_g += region_map[(g, k, 2)][1]
                return r0_g, rows_g

            def _issue_calls(g, gt, itg, r0_g, only_k=None):
                for cl in calls_by_g.get(g, []):
                    k, r0, nr = cl["k"], cl["row0"], cl["rows"]
                    if only_k is not None and k != only_k:
                        continue
                    loc0 = (r0 - r0_g) // P      # tile offset in gt
                    nc.gpsimd.dma_gather(
                        out_ap=gt[:, loc0 * H:(loc0 + nr // P) * H].rearrange(
                            "p (t e) -> p t e", e=H),
                        in_ap=hp_fulls[k][:, :],
                        idxs_ap=itg[:, (r0 - r0_g) // 16:
                                    (r0 - r0_g + nr) // 16],
                        num_idxs=nr, num_idxs_reg=nr, elem_size=H,
                        single_packet=True, queue_num=cl["q"])

            # first WIN groups: issue calls bucket-major across the window
            # so the stream has bucket-k work while collective k+1 flies
            WIN = min(3, cfg.NGRP)
            pre = {}
            for g in range(WIN):
                r0_g, rows_g = _grp_meta(g)
                gt = g1_pool.tile([P, (rows_g // P) * H], F32, tag="g1",
                                  name=f"g1_{g}")
                itg = idx_pool.tile([P, rows_g // 16], I16, tag="idxg",
                                    name=f"itg{g}")
                nc.sync.dma_start(
                    out=itg[:], in_=idx_d[:, r0_g // 16:(r0_g + rows_g) // 16])
                pre[g] = (gt, itg, r0_g, rows_g)
            for k in range(NBKT):
                for g in range(WIN):
                    gt, itg, r0_g, _ = pre[g]
                    _issue_calls(g, gt, itg, r0_g, only_k=k)

            for g in range(cfg.NGRP):
                blocks = list(_group_blocks(cfg, g))
                gb = len(blocks)
                if g in pre:
                    gt, itg, r0_g, rows_g = pre[g]
                else:
                    r0_g, rows_g = _grp_meta(g)
                    gt = g1_pool.tile([P, (rows_g // P) * H], F32, tag="g1",
                                      name=f"g1_{g}")
                    itg = idx_pool.tile([P, rows_g // 16], I16, tag="idxg",
                                        name=f"itg{g}")
                    nc.sync.dma_start(
                        out=itg[:],
                        in_=idx_d[:, r0_g // 16:(r0_g + rows_g) // 16])
                    _issue_calls(g, gt, itg, r0_g)

                # tier-1 reduce per (g, k), combine into zbuf slice
                acc = None
                for k in range(NBKT):
                    r0, nr = region_map[(g, k, 1)]
                    loc0 = (r0 - r0_g) // P
                    pt = part_pool.tile([P, gb * H], F32, tag="part",
                                        name=f"pt{g}_{k}")
                    # gt free layout: slot index s = loc0 + b*S_k + t
                    # reduce over t (innermost AP dim)
                    src_ap = bass.AP(
                        gt.tensor,
                        gt[:].offset + loc0 * H,
                        [gt[:].ap[0],
                         [S[k] * H, gb], [1, H], [H, S[k]]],
                    )
                    nc.vector.tensor_reduce(out=pt[:].rearrange(
                        "p (b e) -> p b e", e=H),
                        in_=src_ap, axis=AX.X, op=OP.add)
                    if acc is None:
                        acc = pt
                    else:
                        nc.vector.tensor_tensor(
                            out=acc[:], in0=acc[:], in1=pt[:], op=OP.add)
                zslice = zbuf[:, blocks[0] * H:(blocks[-1] + 1) * H]
                nc.scalar.activation(out=zslice, in_=acc[:], func=AF.Copy)

                # tier-2 pieces -> psum per block -> add into zbuf
                gp = pieces_by_g.get(g, [])
                by_blk = {}
                for (k, j, blk, ci) in gp:
                    by_blk.setdefault(blk, []).append((k, j, ci))
                OHMAX = 8
                for blk, plist in sorted(by_blk.items()):
                    # pieces of a block are a contiguous ci-run; build its
                    # one-hot matrices in batched stride-0-AP is_equal calls
                    npg = len(plist)
                    ci0 = plist[0][2]
                    assert [ci for (_, _, ci) in plist] == list(
                        range(ci0, ci0 + npg))
                    ps = psB.tile([P, H], F32, tag="ps2", name=f"ps2_{blk}")
                    for c0 in range(0, npg, OHMAX):
                        nch = min(OHMAX, npg - c0)
                        oh = st_pool.tile([P, nch * P], F32, tag="st",
                                          name=f"oh{g}_{blk}_{c0}")
                        in0 = bass.AP(dl2_sb.tensor,
                                      dl2_sb[:].offset + ci0 + c0,
                                      [dl2_sb[:].ap[0], [1, nch], [0, P]])
                        in1 = bass.AP(iota_f.tensor, iota_f[:].offset,
                                      [iota_f[:].ap[0], [0, nch], [1, P]])
                        nc.vector.tensor_tensor(out=oh[:], in0=in0, in1=in1,
                                                op=OP.is_equal)
                        for i in range(nch):
                            k, j, ci = plist[c0 + i]
                            r0, nr = region_map[(g, k, 2)]
                            loc0 = (r0 - r0_g) // P + j
                            nc.tensor.matmul(
                                out=ps[:], lhsT=oh[:, i * P:(i + 1) * P],
                                rhs=gt[:, loc0 * H:(loc0 + 1) * H],
                                start=(c0 + i == 0),
                                stop=(c0 + i == npg - 1))
                    nc.vector.tensor_tensor(
                        out=zbuf[:, blk * H:(blk + 1) * H],
                        in0=zbuf[:, blk * H:(blk + 1) * H],
                        in1=ps[:], op=OP.add)

                # ---- per-group epilogue (overlaps with next group) ----
                b0, b1 = blocks[0], blocks[-1] + 1
                gw = (b1 - b0) * H
                zs = e_pool.tile([P, gw], F32, tag="zs", name=f"zs{g}")
                nc.vector.tensor_tensor(out=zs[:],
                                        in0=zbuf[:, b0 * H:b1 * H],
                                        in1=hp_own[:, b0 * H:b1 * H],
                                        op=OP.add)
                dv = bass.AP(dinv_sb.tensor, dinv_sb[:].offset + b0,
                             [dinv_sb[:].ap[0], [1, b1 - b0], [0, H]])
                nc.vector.tensor_tensor(out=zs[:], in0=zs[:], in1=dv,
                                        op=OP.mult)
                if has_bias:
                    bb = bass.AP(b_sb.tensor, b_sb[:].offset,
                                 [b_sb[:].ap[0], [0, b1 - b0], [1, H]])
                    nc.vector.tensor_tensor(out=zs[:], in0=zs[:], in1=bb,
                                            op=OP.add)
                ta = e_pool.tile([P, gw], F32, tag="ta", name=f"ta{g}")
                nc.vector.tensor_scalar(out=ta[:], in0=zs[:], scalar1=0.0,
                                        scalar2=None, op0=OP.min)
                nc.scalar.activation(out=ta[:], in_=ta[:], func=AF.Exp)
                tb = e_pool.tile([P, gw], F32, tag="tb", name=f"tb{g}")
                nc.vector.tensor_scalar(out=tb[:], in0=zs[:], scalar1=0.0,
                                        scalar2=-1.0, op0=OP.max, op1=OP.add)
                nc.vector.tensor_tensor(out=zs[:], in0=ta[:], in1=tb[:],
                                        op=OP.add)
                for t in range(b0, b1):
                    nc.sync.dma_start(
                        out=out_d[t * P:(t + 1) * P, :],
                        in_=zs[:, (t - b0) * H:(t - b0 + 1) * H])
    nc.compile()
    return nc


def run(cfg, x, W, b, edge_index, **run_kwargs):
    in_maps, meta = _preprocess(cfg, x, W, b, edge_index)
    nc = _build_program(cfg, meta)
    res = bass_utils.run_bass_kernel_spmd(
        nc, in_maps, core_ids=list(range(cfg.C)), **run_kwargs)
    outs = [res.results[c]["out"][:cfg.NPC] for c in range(cfg.C)]
    full = np.concatenate(outs, axis=0).astype(np.float32)
    return full, res


def kernel(x, W, b, edge_index):
    cfg = Cfg(N=100000, E=1600000, F=128, H=64, C=8)
    out, _ = run(cfg, np.asarray(x), np.asarray(W), np.asarray(b),
                 np.asarray(edge_index))
    return out



# revision 24
# speedup vs baseline: 1.1832x; 1.1832x over previous
"""GCN (single GCNConv + ELU) forward on 8 Trainium2 NeuronCores.

out = ELU( D^-1/2 (A + I) D^-1/2 (x @ W) + b )

V2 strategy (1D dst-partition, slot-major tier-1 + one-hot tier-2):
  - Nodes (dst rows) sharded across 8 cores; edges partitioned by dst.
  - Host pre-scales x by dinv[node] so phase A is a pure matmul:
    h' = (x * dinv) @ W, per-core shard, then AllGather -> hp_full in DRAM.
  - The per-edge norm coef dinv[src]*dinv[dst] factors: dinv[src] rides h',
    dinv[dst] is applied once per destination row after aggregation.
  - Tier-1: every dst gets S[k] gather slots per src-bucket (int16 dma_gather
    indices, 4 src buckets of 32768 padded rows). Gathered slot-major:
    G[p=dst, slot, 64]. Segment-sum = ONE strided tensor_reduce per
    (block-group, bucket). Unused slots point at known zero rows of hp_full.
  - Tier-2: excess edges (degree tail) go through a selection-matrix matmul:
    S_T[e, j] = (j == dstloc[e]) built by one tensor_scalar per 128-edge
    piece, psum += S_T.T @ G on the PE per 128-dst block.
  - Epilogue: z = (t1 + t2 + h'_own) * dinv_dst (+ b); out = ELU(z).
  - dma_gather calls are <=1024 rows (single_packet crashes above that),
    round-robined over 4 SWDGE queues (4x faster descriptor generation).

Self-contained: no imports from the problem directory.
"""

import math
import os
import sys

import ml_dtypes
import numpy as np

sys.path.insert(0, "/opt/trn_rl_repo")

import concourse.bacc as bacc  # noqa: E402
import concourse.bass as bass  # noqa: E402
import concourse.library_config as library_config  # noqa: E402
import concourse.mybir as mybir  # noqa: E402
import concourse.tile as tile  # noqa: E402
from concourse import bass_utils  # noqa: E402

F32 = mybir.dt.float32
BF16 = mybir.dt.bfloat16
I16 = mybir.dt.int16
AF = mybir.ActivationFunctionType
OP = mybir.AluOpType
AX = mybir.AxisListType

P = 128
BW = 32768          # src bucket width (int16 gather index limit)
CALL_ROWS = 1024    # max rows per dma_gather call (single_packet limit)
NQ = 4              # SWDGE queues


class Cfg:
    def __init__(self, N, E, F, H, C, GB=5, S=(3, 3, 3, 3), NCHK=4):
        self.N, self.E, self.F, self.H, self.C = N, E, F, H, C
        assert N % C == 0
        self.NPC = N // C
        self.NBLK = math.ceil(self.NPC / P)
        self.NPCPAD = self.NBLK * P
        # chunk-major hp layout: shard split into NCHK chunks; each chunk
        # gets one extra all-zero pad block. Bucket k == chunk k of all
        # cores, so AllGather can be chunked and pipelined with gathers.
        self.NCHK = NCHK
        bpc = math.ceil(self.NBLK / NCHK)           # real blocks per chunk
        self.RBCH = [bpc] * (NCHK - 1) + [self.NBLK - bpc * (NCHK - 1)]
        self.CHB = [rb + 1 for rb in self.RBCH]     # +1 zero block
        cb = [0]
        for w in self.CHB:
            cb.append(cb[-1] + w)
        self.CB = cb                                 # chunk block offsets
        self.SHROWS = cb[-1] * P
        self.NTOTPAD = C * self.SHROWS
        self.NBKT = NCHK
        self.KBASE = [C * cb[k] * P for k in range(NCHK)]
        self.KWID = [C * self.CHB[k] * P for k in range(NCHK)]
        assert all(w <= 32767 for w in self.KWID), self.KWID
        self.GB = GB
        self.NGRP = math.ceil(self.NBLK / GB)
        self.S = list(S)[:self.NBKT]
        if len(self.S) < self.NBKT:
            self.S += [1] * (self.NBKT - len(self.S))

    def src_chunk(self, b):
        """chunk index of a real shard block b."""
        return min(b // (self.RBCH[0]), self.NCHK - 1)


def _group_blocks(cfg, g):
    return range(g * cfg.GB, min((g + 1) * cfg.GB, cfg.NBLK))


def _zero_rows(cfg):
    """Bucket-relative row of core 0's all-zero pad block in each bucket."""
    return {k: cfg.RBCH[k] * P for k in range(cfg.NBKT)}


def _preprocess(cfg, x, W, b, edge_index):
    N, C, NPC, NBLK, NBKT = cfg.N, cfg.C, cfg.NPC, cfg.NBLK, cfg.NBKT
    S = cfg.S
    src = np.asarray(edge_index[0], dtype=np.int64)
    dst = np.asarray(edge_index[1], dtype=np.int64)

    deg = np.bincount(dst, minlength=N).astype(np.float64) + 1.0
    dinv = (1.0 / np.sqrt(deg)).astype(np.float32)
    # chunk-major bucket-relative row of each src node
    sc = src // NPC
    sl = src % NPC
    sb = sl >> 7
    bpc = cfg.RBCH[0]
    sj = np.minimum(sb // bpc, cfg.NCHK - 1)
    chb = np.array(cfg.CHB, dtype=np.int64)
    rel_all = (sc * chb[sj] + (sb - sj * bpc)) * P + (sl & 127)
    bk_all = sj
    zrows = _zero_rows(cfg)
    has_bias = bool(np.any(np.asarray(b) != 0))

    # ---------- per-core structure ----------
    slot_base = np.concatenate([[0], np.cumsum(S)]).astype(np.int64)
    TSLOT = int(slot_base[-1])          # tier-1 slots per dst

    cores = []
    for c in range(C):
        sel = (dst // NPC) == c
        es = rel_all[sel]                   # bucket-relative rows
        ed = (dst[sel] - c * NPC).astype(np.int64)
        bk = bk_all[sel]
        # rank within (dst, bucket)
        order = np.lexsort((es, bk, ed))
        es, ed, bk = es[order], ed[order], bk[order]
        gkey = ed * NBKT + bk
        newseg = np.empty(len(gkey), dtype=bool)
        newseg[0:1] = True
        newseg[1:] = gkey[1:] != gkey[:-1]
        segstart = np.maximum.accumulate(np.where(newseg, np.arange(len(gkey)), 0))
        rank = np.arange(len(gkey)) - segstart
        capk = np.array(S, dtype=np.int64)[bk]
        t1 = rank < capk
        cores.append(dict(es=es, ed=ed, bk=bk, rank=rank, t1=t1))

    # tier-2 tile counts per (group, bucket) must be shared across cores
    t2cnt = np.zeros((C, cfg.NGRP, NBKT), dtype=np.int64)
    for c in range(C):
        d = cores[c]
        m = ~d["t1"]
        gi = d["ed"][m] >> 7
        grp = gi // cfg.GB
        np.add.at(t2cnt[c], (grp, d["bk"][m]), 1)
    T2 = np.ceil(t2cnt / P).astype(np.int64).max(axis=0)    # [NGRP, NBKT] tiles

    # ---------- global row-stream layout (shared) ----------
    # per group g: for k: [tier1: GBcur*S_k*128 rows][tier2: T2[g,k]*128 rows]
    regions = []        # (g, k, kind, row0, nrows)
    row = 0
    for g in range(cfg.NGRP):
        gb = len(_group_blocks(cfg, g))
        for k in range(NBKT):
            n1 = gb * S[k] * P
            regions.append((g, k, 1, row, n1))
            row += n1
            n2 = int(T2[g, k]) * P
            if n2:
                regions.append((g, k, 2, row, n2))
                row += n2
    TOTROWS = row
    region_map = {(g, k, kind): (r0, nr) for (g, k, kind, r0, nr) in regions}

    # gather calls: chunks of <=CALL_ROWS within each region
    calls = []
    for (g, k, kind, r0, nr) in regions:
        off = 0
        while off < nr:
            n = min(CALL_ROWS, nr - off)
            calls.append(dict(g=g, k=k, row0=r0 + off, rows=n,
                              q=len(calls) % NQ))
            off += n

    # tier-2 pieces (shared static structure): for (g,k) tile j, pieces are
    # computed per-core but piece boundaries differ per core! -> make the
    # PIECE LIST static: one piece per (tile, block) for every block in the
    # group: a piece's dstloc column zeroes out non-member rows, so a static
    # piece list valid for all cores is: for each (g,k) tile, all blocks of
    # group g. That would be GB pieces per tile (too many); instead take the
    # union over cores of blocks actually touched by that tile.
    piece_map = {}       # (g,k,j) -> sorted list of blocks
    for c in range(C):
        d = cores[c]
        m = ~d["t1"]
        es2, ed2, bk2 = d["es"][m], d["ed"][m], d["bk"][m]
        o2 = np.lexsort((ed2, bk2, (ed2 >> 7) // cfg.GB))
        es2, ed2, bk2 = es2[o2], ed2[o2], bk2[o2]
        d["t2_sorted"] = (es2, ed2, bk2)
        # positions within (g,k) region
        grp2 = (ed2 >> 7) // cfg.GB
        for (g, k, kind, r0, nr) in regions:
            if kind != 2:
                continue
            mm = (grp2 == g) & (bk2 == k)
            edm = ed2[mm]
            if len(edm) == 0:
                continue
            for j in range(int(T2[g, k])):
                seg = edm[j * P:(j + 1) * P]
                if len(seg) == 0:
                    break
                for blk in np.unique(seg >> 7):
                    piece_map.setdefault((g, k, j), set()).add(int(blk))
    # order pieces by (g, blk, k, j) so each block's pieces are a
    # contiguous ci-run (enables one batched is_equal per block)
    flat = []
    for (g, k, j), blks in sorted(piece_map.items()):
        for blk in sorted(blks):
            flat.append((g, blk, k, j))
    flat.sort()
    piece_list = [(g, k, j, blk, ci)
                  for ci, (g, blk, k, j) in enumerate(flat)]
    NPIECE = len(piece_list)

    # ---------- per-core arrays ----------
    in_maps = []
    xs = np.asarray(x, dtype=np.float32)
    for c in range(C):
        d = cores[c]
        idx_stream = np.zeros(TOTROWS, dtype=np.int16)
        # default: every row points at the zero row of its region's bucket
        for (g, k, kind, r0, nr) in regions:
            idx_stream[r0:r0 + nr] = zrows[k]

        # tier-1 fill
        t1m = d["t1"]
        es1, ed1, bk1, rk1 = (d["es"][t1m], d["ed"][t1m], d["bk"][t1m],
                              d["rank"][t1m])
        blk1 = ed1 >> 7
        g1 = blk1 // cfg.GB
        gb_base = (g1 * cfg.GB)
        b_in_g = blk1 - gb_base
        slot = slot_base[bk1] + rk1          # 0..TSLOT-1 (bucket-sectioned)
        slot_in_k = rk1
        # row = region(g,k,1).row0 + (b_in_g * S_k + slot_in_k)*128 + p
        r0s = np.array([[region_map[(g, k, 1)][0] for k in range(NBKT)]
                        for g in range(cfg.NGRP)], dtype=np.int64)
        Sarr = np.array(S, dtype=np.int64)
        rows1 = (r0s[g1, bk1] + (b_in_g * Sarr[bk1] + slot_in_k) * P
                 + (ed1 & 127))
        idx_stream[rows1] = es1.astype(np.int16)

        # tier-2 fill + dstloc per piece
        dstloc2 = np.full((P, max(NPIECE, 1)), -1.0, dtype=np.float32)
        es2, ed2, bk2 = d["t2_sorted"]
        grp2 = (ed2 >> 7) // cfg.GB
        pos = 0
        pc_index = {(g, k, j, blk): ci for (g, k, j, blk, ci) in piece_list}
        for g in range(cfg.NGRP):
            for k in range(NBKT):
                mm = (grp2 == g) & (bk2 == k)
                nseg = int(mm.sum())
                if (g, k, 2) not in region_map:
                    assert nseg == 0
                    continue
                r0, nr = region_map[(g, k, 2)]
                esg, edg = es2[mm], ed2[mm]
                assert nseg <= nr
                idx_stream[r0:r0 + nseg] = esg.astype(np.int16)
                for j in range(int(T2[g, k])):
                    seg_ed = edg[j * P:(j + 1) * P]
                    if len(seg_ed) == 0:
                        break
                    for blk in np.unique(seg_ed >> 7):
                        ci = pc_index[(g, k, j, int(blk))]
                        rows_in_tile = np.nonzero((seg_ed >> 7) == blk)[0]
                        dstloc2[rows_in_tile, ci] = (seg_ed[rows_in_tile]
                                                     & 127).astype(np.float32)
        # wrapped idx per call, replicated to 128 partitions
        idx16 = np.zeros((16, TOTROWS // 16), dtype=np.int16)
        for cl in calls:
            r0, nr = cl["row0"], cl["rows"]
            idx16[:, r0 // 16:(r0 + nr) // 16] = \
                idx_stream[r0:r0 + nr].reshape(-1, 16).T
        idx_dram = np.tile(idx16, (8, 1))

        xc = np.zeros((cfg.NPCPAD, cfg.F), dtype=np.float32)
        xc[:NPC] = xs[c * NPC:(c + 1) * NPC] * dinv[c * NPC:(c + 1) * NPC, None]
        dv = np.zeros(cfg.NPCPAD, dtype=np.float32)
        dv[:NPC] = dinv[c * NPC:(c + 1) * NPC]
        m = {
            "iotaf": np.broadcast_to(
                np.arange(P, dtype=np.float32), (P, P)).copy(),
            "xT": np.ascontiguousarray(xc.T).astype(ml_dtypes.bfloat16),
            "Wm": np.ascontiguousarray(W, dtype=np.float32).astype(
                ml_dtypes.bfloat16),
            "dinv_own": np.ascontiguousarray(dv.reshape(NBLK, P).T),
            "dstloc2": np.ascontiguousarray(dstloc2),
            "idx": idx_dram,
        }
        if has_bias:
            m["bvec"] = np.ascontiguousarray(
                np.broadcast_to(np.asarray(b, np.float32), (P, cfg.H)))
        in_maps.append(m)

    meta = dict(T2=T2, regions=regions, region_map=region_map, calls=calls,
                piece_list=piece_list, NPIECE=NPIECE, TOTROWS=TOTROWS,
                has_bias=has_bias, TSLOT=TSLOT)
    return in_maps, meta


def _build_program(cfg, meta):
    C, H, F, NBLK, NBKT = cfg.C, cfg.H, cfg.F, cfg.NBLK, cfg.NBKT
    S = cfg.S
    T2 = meta["T2"]
    region_map = meta["region_map"]
    calls = meta["calls"]
    piece_list = meta["piece_list"]
    NPIECE = meta["NPIECE"]
    TOTROWS = meta["TOTROWS"]
    has_bias = meta["has_bias"]
    TSLOT = meta["TSLOT"]

    nc = bacc.Bacc(num_devices=C, num_swdge_queues=NQ)
    xT = nc.dram_tensor("xT", [F, cfg.NPCPAD], BF16, kind="ExternalInput")
    Wm = nc.dram_tensor("Wm", [F, H], BF16, kind="ExternalInput")
    dinv_d = nc.dram_tensor("dinv_own", [P, NBLK], F32, kind="ExternalInput")
    dl2_d = nc.dram_tensor("dstloc2", [P, max(NPIECE, 1)], F32,
                           kind="ExternalInput")
    idx_d = nc.dram_tensor("idx", [P, TOTROWS // 16], I16, kind="ExternalInput")
    iota_d = nc.dram_tensor("iotaf", [P, P], F32, kind="ExternalInput")
    if has_bias:
        bvec_d = nc.dram_tensor("bvec", [P, H], F32, kind="ExternalInput")
    out_d = nc.dram_tensor("out", [cfg.NPCPAD, H], F32, kind="ExternalOutput")

    calls_by_g = {}
    for cl in calls:
        calls_by_g.setdefault(cl["g"], []).append(cl)
    pieces_by_g = {}
    for (g, k, j, blk, ci) in piece_list:
        pieces_by_g.setdefault(g, []).append((k, j, blk, ci))

    with tile.TileContext(nc, num_cores=C) as tc:
        with (
            tc.tile_pool(name="const", bufs=1) as cpool,
            tc.tile_pool(name="dram", bufs=1, space="DRAM") as dram,
            tc.tile_pool(name="xa", bufs=3) as xa_pool,
            tc.tile_pool(name="psA", bufs=1, space="PSUM") as psA,
            tc.tile_pool(name="psB", bufs=4, space="PSUM") as psB,
            tc.tile_pool(name="st", bufs=2) as st_pool,
            tc.tile_pool(name="g1", bufs=3) as g1_pool,
            tc.tile_pool(name="part", bufs=3) as part_pool,
            tc.tile_pool(name="idxp", bufs=2) as idx_pool,
            tc.tile_pool(name="ep", bufs=2) as e_pool,
        ):
            nc.gpsimd.load_library(library_config.mlp)
            W_sb = cpool.tile([F, H], BF16)
            nc.sync.dma_start(out=W_sb[:], in_=Wm[:, :])
            dinv_sb = cpool.tile([P, NBLK], F32)
            nc.sync.dma_start(out=dinv_sb[:], in_=dinv_d[:, :])
            dl2_sb = cpool.tile([P, max(NPIECE, 1)], F32)
            nc.sync.dma_start(out=dl2_sb[:], in_=dl2_d[:, :])
            iota_f = cpool.tile([P, P], F32)
            nc.sync.dma_start(out=iota_f[:], in_=iota_d[:, :])
            if has_bias:
                b_sb = cpool.tile([P, H], F32)
                nc.sync.dma_start(out=b_sb[:], in_=bvec_d[:, :])
            hp_own = cpool.tile([P, NBLK * H], F32)
            zbuf = cpool.tile([P, NBLK * H], F32)

            # ---- phase A: h' shard, chunk-major, pipelined AllGathers ----
            # one DRAM tensor per chunk so Tile dependencies let bucket-k
            # gathers start right after collective k (not the whole gather)
            hp_shards = [dram.tile([cfg.CHB[k] * P, H], F32,
                                   name=f"hpsh{k}") for k in range(NBKT)]
            hp_fulls = [dram.tile([cfg.KWID[k], H], F32, addr_space="Shared",
                                  name=f"hpf{k}") for k in range(NBKT)]
            zt = xa_pool.tile([P, H], F32, tag="zt")
            nc.vector.memset(zt[:], 0.0)
            bigp = psA.tile([P, 2048], F32)
            bpc = cfg.RBCH[0]
            XB = 4                           # x blocks per load
            for k in range(NBKT):
                tlo = k * bpc
                nb = cfg.RBCH[k]             # real blocks in this chunk
                for i0 in range(0, nb, XB):
                    nx = min(XB, nb - i0)
                    xt = xa_pool.tile([P, nx * P], BF16, tag="xt",
                                      name=f"xt{k}_{i0}")
                    nc.sync.dma_start(
                        out=xt[:],
                        in_=xT[:, (tlo + i0) * P:(tlo + i0 + nx) * P])
                    for i in range(nx):
                        j = i0 + i
                        nc.tensor.matmul(out=bigp[:, j * H:(j + 1) * H],
                                         lhsT=xt[:, i * P:(i + 1) * P],
                                         rhs=W_sb[:], start=True, stop=True)
                nc.scalar.activation(
                    out=hp_own[:, tlo * H:(tlo + nb) * H],
                    in_=bigp[:, :nb * H], func=AF.Copy)
                # one batched store: SBUF [p, (lb, h)] -> DRAM [(lb, p), h]
                st_in = bass.AP(hp_own.tensor, hp_own[:].offset + tlo * H,
                                [hp_own[:].ap[0], [H, nb], [1, H]])
                st_out = bass.AP(hp_shards[k].tensor,
                                 hp_shards[k][:].offset,
                                 [[H, P], [P * H, nb], [1, H]])
                nc.sync.dma_start(out=st_out, in_=st_in)
                zb = cfg.RBCH[k]             # zero block position
                nc.sync.dma_start(out=hp_shards[k][zb * P:(zb + 1) * P, :],
                                  in_=zt[:])
                nc.gpsimd.collective_compute(
                    "AllGather", OP.bypass,
                    replica_groups=[list(range(C))],
                    ins=[hp_shards[k][:, :]], outs=[hp_fulls[k][:, :]])

            # ---- phase B ----
            for g in range(cfg.NGRP):
                blocks = list(_group_blocks(cfg, g))
                gb = len(blocks)
                # one SBUF buffer per (g): tier1 rows then tier2 rows
                r0_g = region_map[(g, 0, 1)][0]
                rows_g = 0
                for k in range(NBKT):
                    rows_g += region_map[(g, k, 1)][1]
                    if (g, k, 2) in region_map:
                        rows_g += region_map[(g, k, 2)][1]
                gt = g1_pool.tile([P, (rows_g // P) * H], F32, tag="g1",
                                  name=f"g1_{g}")
                itg = idx_pool.tile([P, rows_g // 16], I16, tag="idxg",
                                    name=f"itg{g}")
                nc.sync.dma_start(
                    out=itg[:], in_=idx_d[:, r0_g // 16:(r0_g + rows_g) // 16])
                for cl in calls_by_g.get(g, []):
                    k, r0, nr = cl["k"], cl["row0"], cl["rows"]
                    loc0 = (r0 - r0_g) // P          # tile offset in gt
                    nc.gpsimd.dma_gather(
                        out_ap=gt[:, loc0 * H:(loc0 + nr // P) * H].rearrange(
                            "p (t e) -> p t e", e=H),
                        in_ap=hp_fulls[k][:, :],
                        idxs_ap=itg[:, (r0 - r0_g) // 16:(r0 - r0_g + nr) // 16],
                        num_idxs=nr, num_idxs_reg=nr, elem_size=H,
                        single_packet=True, queue_num=cl["q"])

                # tier-1 reduce per (g, k), combine into zbuf slice
                acc = None
                for k in range(NBKT):
                    r0, nr = region_map[(g, k, 1)]
                    loc0 = (r0 - r0_g) // P
                    pt = part_pool.tile([P, gb * H], F32, tag="part",
                                        name=f"pt{g}_{k}")
                    # gt free layout: slot index s = loc0 + b*S_k + t
                    # reduce over t (innermost AP dim)
                    src_ap = bass.AP(
                        gt.tensor,
                        gt[:].offset + loc0 * H,
                        [gt[:].ap[0],
                         [S[k] * H, gb], [1, H], [H, S[k]]],
                    )
                    nc.vector.tensor_reduce(out=pt[:].rearrange(
                        "p (b e) -> p b e", e=H),
                        in_=src_ap, axis=AX.X, op=OP.add)
                    if acc is None:
                        acc = pt
                    else:
                        nc.vector.tensor_tensor(
                            out=acc[:], in0=acc[:], in1=pt[:], op=OP.add)
                zslice = zbuf[:, blocks[0] * H:(blocks[-1] + 1) * H]
                nc.scalar.activation(out=zslice, in_=acc[:], func=AF.Copy)

                # tier-2 pieces -> psum per block -> add into zbuf
                gp = pieces_by_g.get(g, [])
                by_blk = {}
                for (k, j, blk, ci) in gp:
                    by_blk.setdefault(blk, []).append((k, j, ci))
                OHMAX = 8
                for blk, plist in sorted(by_blk.items()):
                    # pieces of a block are a contiguous ci-run; build its
                    # one-hot matrices in batched stride-0-AP is_equal calls
                    npg = len(plist)
                    ci0 = plist[0][2]
                    assert [ci for (_, _, ci) in plist] == list(
                        range(ci0, ci0 + npg))
                    ps = psB.tile([P, H], F32, tag="ps2", name=f"ps2_{blk}")
                    for c0 in range(0, npg, OHMAX):
                        nch = min(OHMAX, npg - c0)
                        oh = st_pool.tile([P, nch * P], F32, tag="st",
                                          name=f"oh{g}_{blk}_{c0}")
                        in0 = bass.AP(dl2_sb.tensor,
                                      dl2_sb[:].offset + ci0 + c0,
                                      [dl2_sb[:].ap[0], [1, nch], [0, P]])
                        in1 = bass.AP(iota_f.tensor, iota_f[:].offset,
                                      [iota_f[:].ap[0], [0, nch], [1, P]])
                        nc.vector.tensor_tensor(out=oh[:], in0=in0, in1=in1,
                                                op=OP.is_equal)
                        for i in range(nch):
                            k, j, ci = plist[c0 + i]
                            r0, nr = region_map[(g, k, 2)]
                            loc0 = (r0 - r0_g) // P + j
                            nc.tensor.matmul(
                                out=ps[:], lhsT=oh[:, i * P:(i + 1) * P],
                                rhs=gt[:, loc0 * H:(loc0 + 1) * H],
                                start=(c0 + i == 0),
                                stop=(c0 + i == npg - 1))
                    nc.vector.tensor_tensor(
                        out=zbuf[:, blk * H:(blk + 1) * H],
                        in0=zbuf[:, blk * H:(blk + 1) * H],
                        in1=ps[:], op=OP.add)

                # ---- per-group epilogue (overlaps with next group) ----
                b0, b1 = blocks[0], blocks[-1] + 1
                gw = (b1 - b0) * H
                zs = e_pool.tile([P, gw], F32, tag="zs", name=f"zs{g}")
                nc.vector.tensor_tensor(out=zs[:],
                                        in0=zbuf[:, b0 * H:b1 * H],
                                        in1=hp_own[:, b0 * H:b1 * H],
                                        op=OP.add)
                dv = bass.AP(dinv_sb.tensor, dinv_sb[:].offset + b0,
                             [dinv_sb[:].ap[0], [1, b1 - b0], [0, H]])
                nc.vector.tensor_tensor(out=zs[:], in0=zs[:], in1=dv,
                                        op=OP.mult)
                if has_bias:
                    bb = bass.AP(b_sb.tensor, b_sb[:].offset,
                                 [b_sb[:].ap[0], [0, b1 - b0], [1, H]])
                    nc.vector.tensor_tensor(out=zs[:], in0=zs[:], in1=bb,
                                            op=OP.add)
                ta = e_pool.tile([P, gw], F32, tag="ta", name=f"ta{g}")
                nc.vector.tensor_scalar(out=ta[:], in0=zs[:], scalar1=0.0,
                                        scalar2=None, op0=OP.min)
                nc.scalar.activation(out=ta[:], in_=ta[:], func=AF.Exp)
                tb = e_pool.tile([P, gw], F32, tag="tb", name=f"tb{g}")
                nc.vector.tensor_scalar(out=tb[:], in0=zs[:], scalar1=0.0,
                                        scalar2=-1.0, op0=OP.max, op1=OP.add)
                nc.vector.tensor_tensor(out=zs[:], in0=ta[:], in1=tb[:],
                                        op=OP.add)
                so = bass.AP(out_d[:, :].tensor,
                             out_d[:, :].offset + b0 * P * H,
                             [[H, P], [P * H, b1 - b0], [1, H]])
                si = bass.AP(zs.tensor, zs[:].offset,
                             [zs[:].ap[0], [H, b1 - b0], [1, H]])
                nc.sync.dma_start(out=so, in_=si)
    nc.compile()
    return nc


def run(cfg, x, W, b, edge_index, **run_kwargs):
    in_maps, meta = _preprocess(cfg, x, W, b, edge_index)
    nc = _build_program(cfg, meta)
    res = bass_utils.run_bass_kernel_spmd(
        nc, in_maps, core_ids=list(range(cfg.C)), **run_kwargs)
    outs = [res.results[c]["out"][:cfg.NPC] for c in range(cfg.C)]
    full = np.concatenate(outs, axis=0).astype(np.float32)
    return full, res


def kernel(x, W, b, edge_index):
    cfg = Cfg(N=100000, E=1600000, F=128, H=64, C=8)
    out, _ = run(cfg, np.asarray(x), np.asarray(W), np.asarray(b),
                 np.asarray(edge_index))
    return out



# revision 26
# speedup vs baseline: 1.1996x; 1.0139x over previous
"""GCN (single GCNConv + ELU) forward on 8 Trainium2 NeuronCores.

out = ELU( D^-1/2 (A + I) D^-1/2 (x @ W) + b )

V2 strategy (1D dst-partition, slot-major tier-1 + one-hot tier-2):
  - Nodes (dst rows) sharded across 8 cores; edges partitioned by dst.
  - Host pre-scales x by dinv[node] so phase A is a pure matmul:
    h' = (x * dinv) @ W, per-core shard, then AllGather -> hp_full in DRAM.
  - The per-edge norm coef dinv[src]*dinv[dst] factors: dinv[src] rides h',
    dinv[dst] is applied once per destination row after aggregation.
  - Tier-1: every dst gets S[k] gather slots per src-bucket (int16 dma_gather
    indices, 4 src buckets of 32768 padded rows). Gathered slot-major:
    G[p=dst, slot, 64]. Segment-sum = ONE strided tensor_reduce per
    (block-group, bucket). Unused slots point at known zero rows of hp_full.
  - Tier-2: excess edges (degree tail) go through a selection-matrix matmul:
    S_T[e, j] = (j == dstloc[e]) built by one tensor_scalar per 128-edge
    piece, psum += S_T.T @ G on the PE per 128-dst block.
  - Epilogue: z = (t1 + t2 + h'_own) * dinv_dst (+ b); out = ELU(z).
  - dma_gather calls are <=1024 rows (single_packet crashes above that),
    round-robined over 4 SWDGE queues (4x faster descriptor generation).

Self-contained: no imports from the problem directory.
"""

import math
import os
import sys

import ml_dtypes
import numpy as np

sys.path.insert(0, "/opt/trn_rl_repo")

import concourse.bacc as bacc  # noqa: E402
import concourse.bass as bass  # noqa: E402
import concourse.library_config as library_config  # noqa: E402
import concourse.mybir as mybir  # noqa: E402
import concourse.tile as tile  # noqa: E402
from concourse import bass_utils  # noqa: E402

F32 = mybir.dt.float32
BF16 = mybir.dt.bfloat16
I16 = mybir.dt.int16
AF = mybir.ActivationFunctionType
OP = mybir.AluOpType
AX = mybir.AxisListType

P = 128
BW = 32768          # src bucket width (int16 gather index limit)
CALL_ROWS = 1024    # max rows per dma_gather call (single_packet limit)
NQ = 4              # SWDGE queues


class Cfg:
    def __init__(self, N, E, F, H, C, GB=5, S=(3, 3, 3, 3), NCHK=4):
        self.N, self.E, self.F, self.H, self.C = N, E, F, H, C
        assert N % C == 0
        self.NPC = N // C
        self.NBLK = math.ceil(self.NPC / P)
        self.NPCPAD = self.NBLK * P
        # chunk-major hp layout: shard split into NCHK chunks; each chunk
        # gets one extra all-zero pad block. Bucket k == chunk k of all
        # cores, so AllGather can be chunked and pipelined with gathers.
        self.NCHK = NCHK
        bpc = math.ceil(self.NBLK / NCHK)           # real blocks per chunk
        self.RBCH = [bpc] * (NCHK - 1) + [self.NBLK - bpc * (NCHK - 1)]
        self.CHB = [rb + 1 for rb in self.RBCH]     # +1 zero block
        cb = [0]
        for w in self.CHB:
            cb.append(cb[-1] + w)
        self.CB = cb                                 # chunk block offsets
        self.SHROWS = cb[-1] * P
        self.NTOTPAD = C * self.SHROWS
        self.NBKT = NCHK
        self.KBASE = [C * cb[k] * P for k in range(NCHK)]
        self.KWID = [C * self.CHB[k] * P for k in range(NCHK)]
        assert all(w <= 32767 for w in self.KWID), self.KWID
        self.GB = GB
        self.NGRP = math.ceil(self.NBLK / GB)
        self.S = list(S)[:self.NBKT]
        if len(self.S) < self.NBKT:
            self.S += [1] * (self.NBKT - len(self.S))

    def src_chunk(self, b):
        """chunk index of a real shard block b."""
        return min(b // (self.RBCH[0]), self.NCHK - 1)


def _group_blocks(cfg, g):
    return range(g * cfg.GB, min((g + 1) * cfg.GB, cfg.NBLK))


def _zero_rows(cfg):
    """Bucket-relative row of core 0's all-zero pad block in each bucket."""
    return {k: cfg.RBCH[k] * P for k in range(cfg.NBKT)}


def _preprocess(cfg, x, W, b, edge_index):
    N, C, NPC, NBLK, NBKT = cfg.N, cfg.C, cfg.NPC, cfg.NBLK, cfg.NBKT
    S = cfg.S
    src = np.asarray(edge_index[0], dtype=np.int64)
    dst = np.asarray(edge_index[1], dtype=np.int64)

    deg = np.bincount(dst, minlength=N).astype(np.float64) + 1.0
    dinv = (1.0 / np.sqrt(deg)).astype(np.float32)
    # chunk-major bucket-relative row of each src node
    sc = src // NPC
    sl = src % NPC
    sb = sl >> 7
    bpc = cfg.RBCH[0]
    sj = np.minimum(sb // bpc, cfg.NCHK - 1)
    chb = np.array(cfg.CHB, dtype=np.int64)
    rel_all = (sc * chb[sj] + (sb - sj * bpc)) * P + (sl & 127)
    bk_all = sj
    zrows = _zero_rows(cfg)
    has_bias = bool(np.any(np.asarray(b) != 0))

    # ---------- per-core structure ----------
    slot_base = np.concatenate([[0], np.cumsum(S)]).astype(np.int64)
    TSLOT = int(slot_base[-1])          # tier-1 slots per dst

    cores = []
    for c in range(C):
        sel = (dst // NPC) == c
        es = rel_all[sel]                   # bucket-relative rows
        ed = (dst[sel] - c * NPC).astype(np.int64)
        bk = bk_all[sel]
        # rank within (dst, bucket)
        order = np.lexsort((es, bk, ed))
        es, ed, bk = es[order], ed[order], bk[order]
        gkey = ed * NBKT + bk
        newseg = np.empty(len(gkey), dtype=bool)
        newseg[0:1] = True
        newseg[1:] = gkey[1:] != gkey[:-1]
        segstart = np.maximum.accumulate(np.where(newseg, np.arange(len(gkey)), 0))
        rank = np.arange(len(gkey)) - segstart
        capk = np.array(S, dtype=np.int64)[bk]
        t1 = rank < capk
        cores.append(dict(es=es, ed=ed, bk=bk, rank=rank, t1=t1))

    # tier-2 tile counts per (group, bucket) must be shared across cores
    t2cnt = np.zeros((C, cfg.NGRP, NBKT), dtype=np.int64)
    for c in range(C):
        d = cores[c]
        m = ~d["t1"]
        gi = d["ed"][m] >> 7
        grp = gi // cfg.GB
        np.add.at(t2cnt[c], (grp, d["bk"][m]), 1)
    T2 = np.ceil(t2cnt / P).astype(np.int64).max(axis=0)    # [NGRP, NBKT] tiles

    # ---------- global row-stream layout (shared) ----------
    # per group g: for k: [tier1: GBcur*S_k*128 rows][tier2: T2[g,k]*128 rows]
    regions = []        # (g, k, kind, row0, nrows)
    row = 0
    for g in range(cfg.NGRP):
        gb = len(_group_blocks(cfg, g))
        for k in range(NBKT):
            n1 = gb * S[k] * P
            regions.append((g, k, 1, row, n1))
            row += n1
            n2 = int(T2[g, k]) * P
            if n2:
                regions.append((g, k, 2, row, n2))
                row += n2
    TOTROWS = row
    region_map = {(g, k, kind): (r0, nr) for (g, k, kind, r0, nr) in regions}

    # gather calls: chunks of <=CALL_ROWS within each region
    calls = []
    for (g, k, kind, r0, nr) in regions:
        off = 0
        while off < nr:
            n = min(CALL_ROWS, nr - off)
            calls.append(dict(g=g, k=k, row0=r0 + off, rows=n,
                              q=len(calls) % NQ))
            off += n

    # tier-2 pieces (shared static structure): for (g,k) tile j, pieces are
    # computed per-core but piece boundaries differ per core! -> make the
    # PIECE LIST static: one piece per (tile, block) for every block in the
    # group: a piece's dstloc column zeroes out non-member rows, so a static
    # piece list valid for all cores is: for each (g,k) tile, all blocks of
    # group g. That would be GB pieces per tile (too many); instead take the
    # union over cores of blocks actually touched by that tile.
    piece_map = {}       # (g,k,j) -> sorted list of blocks
    for c in range(C):
        d = cores[c]
        m = ~d["t1"]
        es2, ed2, bk2 = d["es"][m], d["ed"][m], d["bk"][m]
        o2 = np.lexsort((ed2, bk2, (ed2 >> 7) // cfg.GB))
        es2, ed2, bk2 = es2[o2], ed2[o2], bk2[o2]
        d["t2_sorted"] = (es2, ed2, bk2)
        # positions within (g,k) region
        grp2 = (ed2 >> 7) // cfg.GB
        for (g, k, kind, r0, nr) in regions:
            if kind != 2:
                continue
            mm = (grp2 == g) & (bk2 == k)
            edm = ed2[mm]
            if len(edm) == 0:
                continue
            for j in range(int(T2[g, k])):
                seg = edm[j * P:(j + 1) * P]
                if len(seg) == 0:
                    break
                for blk in np.unique(seg >> 7):
                    piece_map.setdefault((g, k, j), set()).add(int(blk))
    # order pieces by (g, blk, k, j) so each block's pieces are a
    # contiguous ci-run (enables one batched is_equal per block)
    flat = []
    for (g, k, j), blks in sorted(piece_map.items()):
        for blk in sorted(blks):
            flat.append((g, blk, k, j))
    flat.sort()
    piece_list = [(g, k, j, blk, ci)
                  for ci, (g, blk, k, j) in enumerate(flat)]
    NPIECE = len(piece_list)

    # ---------- per-core arrays ----------
    in_maps = []
    xs = np.asarray(x, dtype=np.float32)
    for c in range(C):
        d = cores[c]
        idx_stream = np.zeros(TOTROWS, dtype=np.int16)
        # default: every row points at the zero row of its region's bucket
        for (g, k, kind, r0, nr) in regions:
            idx_stream[r0:r0 + nr] = zrows[k]

        # tier-1 fill
        t1m = d["t1"]
        es1, ed1, bk1, rk1 = (d["es"][t1m], d["ed"][t1m], d["bk"][t1m],
                              d["rank"][t1m])
        blk1 = ed1 >> 7
        g1 = blk1 // cfg.GB
        gb_base = (g1 * cfg.GB)
        b_in_g = blk1 - gb_base
        slot = slot_base[bk1] + rk1          # 0..TSLOT-1 (bucket-sectioned)
        slot_in_k = rk1
        # row = region(g,k,1).row0 + (b_in_g * S_k + slot_in_k)*128 + p
        r0s = np.array([[region_map[(g, k, 1)][0] for k in range(NBKT)]
                        for g in range(cfg.NGRP)], dtype=np.int64)
        Sarr = np.array(S, dtype=np.int64)
        rows1 = (r0s[g1, bk1] + (b_in_g * Sarr[bk1] + slot_in_k) * P
                 + (ed1 & 127))
        idx_stream[rows1] = es1.astype(np.int16)

        # tier-2 fill + dstloc per piece
        dstloc2 = np.full((P, max(NPIECE, 1)), -1.0, dtype=np.float32)
        es2, ed2, bk2 = d["t2_sorted"]
        grp2 = (ed2 >> 7) // cfg.GB
        pos = 0
        pc_index = {(g, k, j, blk): ci for (g, k, j, blk, ci) in piece_list}
        for g in range(cfg.NGRP):
            for k in range(NBKT):
                mm = (grp2 == g) & (bk2 == k)
                nseg = int(mm.sum())
                if (g, k, 2) not in region_map:
                    assert nseg == 0
                    continue
                r0, nr = region_map[(g, k, 2)]
                esg, edg = es2[mm], ed2[mm]
                assert nseg <= nr
                idx_stream[r0:r0 + nseg] = esg.astype(np.int16)
                for j in range(int(T2[g, k])):
                    seg_ed = edg[j * P:(j + 1) * P]
                    if len(seg_ed) == 0:
                        break
                    for blk in np.unique(seg_ed >> 7):
                        ci = pc_index[(g, k, j, int(blk))]
                        rows_in_tile = np.nonzero((seg_ed >> 7) == blk)[0]
                        dstloc2[rows_in_tile, ci] = (seg_ed[rows_in_tile]
                                                     & 127).astype(np.float32)
        # wrapped idx per call, replicated to 128 partitions
        idx16 = np.zeros((16, TOTROWS // 16), dtype=np.int16)
        for cl in calls:
            r0, nr = cl["row0"], cl["rows"]
            idx16[:, r0 // 16:(r0 + nr) // 16] = \
                idx_stream[r0:r0 + nr].reshape(-1, 16).T
        idx_dram = np.tile(idx16, (8, 1))

        xc = np.zeros((cfg.NPCPAD, cfg.F), dtype=np.float32)
        xc[:NPC] = xs[c * NPC:(c + 1) * NPC] * dinv[c * NPC:(c + 1) * NPC, None]
        dv = np.zeros(cfg.NPCPAD, dtype=np.float32)
        dv[:NPC] = dinv[c * NPC:(c + 1) * NPC]
        m = {
            "iotaf": np.broadcast_to(
                np.arange(P, dtype=np.float32), (P, P)).copy(),
            "xT": np.ascontiguousarray(xc.T).astype(ml_dtypes.bfloat16),
            "Wm": np.ascontiguousarray(W, dtype=np.float32).astype(
                ml_dtypes.bfloat16),
            "dinv_own": np.ascontiguousarray(dv.reshape(NBLK, P).T),
            "dstloc2": np.ascontiguousarray(dstloc2),
            "idx": idx_dram,
        }
        if has_bias:
            m["bvec"] = np.ascontiguousarray(
                np.broadcast_to(np.asarray(b, np.float32), (P, cfg.H)))
        in_maps.append(m)

    meta = dict(T2=T2, regions=regions, region_map=region_map, calls=calls,
                piece_list=piece_list, NPIECE=NPIECE, TOTROWS=TOTROWS,
                has_bias=has_bias, TSLOT=TSLOT)
    return in_maps, meta


def _build_program(cfg, meta):
    C, H, F, NBLK, NBKT = cfg.C, cfg.H, cfg.F, cfg.NBLK, cfg.NBKT
    S = cfg.S
    T2 = meta["T2"]
    region_map = meta["region_map"]
    calls = meta["calls"]
    piece_list = meta["piece_list"]
    NPIECE = meta["NPIECE"]
    TOTROWS = meta["TOTROWS"]
    has_bias = meta["has_bias"]
    TSLOT = meta["TSLOT"]

    nc = bacc.Bacc(num_devices=C, num_swdge_queues=NQ)
    xT = nc.dram_tensor("xT", [F, cfg.NPCPAD], BF16, kind="ExternalInput")
    Wm = nc.dram_tensor("Wm", [F, H], BF16, kind="ExternalInput")
    dinv_d = nc.dram_tensor("dinv_own", [P, NBLK], F32, kind="ExternalInput")
    dl2_d = nc.dram_tensor("dstloc2", [P, max(NPIECE, 1)], F32,
                           kind="ExternalInput")
    idx_d = nc.dram_tensor("idx", [P, TOTROWS // 16], I16, kind="ExternalInput")
    iota_d = nc.dram_tensor("iotaf", [P, P], F32, kind="ExternalInput")
    if has_bias:
        bvec_d = nc.dram_tensor("bvec", [P, H], F32, kind="ExternalInput")
    out_d = nc.dram_tensor("out", [cfg.NPCPAD, H], F32, kind="ExternalOutput")

    calls_by_g = {}
    for cl in calls:
        calls_by_g.setdefault(cl["g"], []).append(cl)
    pieces_by_g = {}
    for (g, k, j, blk, ci) in piece_list:
        pieces_by_g.setdefault(g, []).append((k, j, blk, ci))

    with tile.TileContext(nc, num_cores=C) as tc:
        with (
            tc.tile_pool(name="const", bufs=1) as cpool,
            tc.tile_pool(name="dram", bufs=1, space="DRAM") as dram,
            tc.tile_pool(name="xa", bufs=3) as xa_pool,
            tc.tile_pool(name="psA", bufs=1, space="PSUM") as psA,
            tc.tile_pool(name="psB", bufs=4, space="PSUM") as psB,
            tc.tile_pool(name="st", bufs=2) as st_pool,
            tc.tile_pool(name="g1", bufs=4) as g1_pool,
            tc.tile_pool(name="part", bufs=3) as part_pool,
            tc.tile_pool(name="idxp", bufs=2) as idx_pool,
            tc.tile_pool(name="ep", bufs=2) as e_pool,
        ):
            nc.gpsimd.load_library(library_config.mlp)
            W_sb = cpool.tile([F, H], BF16)
            nc.sync.dma_start(out=W_sb[:], in_=Wm[:, :])
            dinv_sb = cpool.tile([P, NBLK], F32)
            nc.sync.dma_start(out=dinv_sb[:], in_=dinv_d[:, :])
            dl2_sb = cpool.tile([P, max(NPIECE, 1)], F32)
            nc.sync.dma_start(out=dl2_sb[:], in_=dl2_d[:, :])
            iota_f = cpool.tile([P, P], F32)
            nc.sync.dma_start(out=iota_f[:], in_=iota_d[:, :])
            if has_bias:
                b_sb = cpool.tile([P, H], F32)
                nc.sync.dma_start(out=b_sb[:], in_=bvec_d[:, :])
            hp_own = cpool.tile([P, NBLK * H], F32)
            zbuf = cpool.tile([P, NBLK * H], F32)

            # ---- phase A: h' shard, chunk-major, pipelined AllGathers ----
            # one DRAM tensor per chunk so Tile dependencies let bucket-k
            # gathers start right after collective k (not the whole gather)
            hp_shards = [dram.tile([cfg.CHB[k] * P, H], F32,
                                   name=f"hpsh{k}") for k in range(NBKT)]
            hp_fulls = [dram.tile([cfg.KWID[k], H], F32, addr_space="Shared",
                                  name=f"hpf{k}") for k in range(NBKT)]
            zt = xa_pool.tile([P, H], F32, tag="zt")
            nc.vector.memset(zt[:], 0.0)
            bigp = psA.tile([P, 2048], F32)
            bpc = cfg.RBCH[0]
            XB = 4                           # x blocks per load
            for k in range(NBKT):
                tlo = k * bpc
                nb = cfg.RBCH[k]             # real blocks in this chunk
                for i0 in range(0, nb, XB):
                    nx = min(XB, nb - i0)
                    xt = xa_pool.tile([P, nx * P], BF16, tag="xt",
                                      name=f"xt{k}_{i0}")
                    nc.sync.dma_start(
                        out=xt[:],
                        in_=xT[:, (tlo + i0) * P:(tlo + i0 + nx) * P])
                    for i in range(nx):
                        j = i0 + i
                        nc.tensor.matmul(out=bigp[:, j * H:(j + 1) * H],
                                         lhsT=xt[:, i * P:(i + 1) * P],
                                         rhs=W_sb[:], start=True, stop=True)
                nc.scalar.activation(
                    out=hp_own[:, tlo * H:(tlo + nb) * H],
                    in_=bigp[:, :nb * H], func=AF.Copy)
                # one batched store: SBUF [p, (lb, h)] -> DRAM [(lb, p), h]
                st_in = bass.AP(hp_own.tensor, hp_own[:].offset + tlo * H,
                                [hp_own[:].ap[0], [H, nb], [1, H]])
                st_out = bass.AP(hp_shards[k].tensor,
                                 hp_shards[k][:].offset,
                                 [[H, P], [P * H, nb], [1, H]])
                nc.sync.dma_start(out=st_out, in_=st_in)
                zb = cfg.RBCH[k]             # zero block position
                nc.sync.dma_start(out=hp_shards[k][zb * P:(zb + 1) * P, :],
                                  in_=zt[:])
                nc.gpsimd.collective_compute(
                    "AllGather", OP.bypass,
                    replica_groups=[list(range(C))],
                    ins=[hp_shards[k][:, :]], outs=[hp_fulls[k][:, :]])

            # ---- phase B ----
            def _grp_meta(g):
                r0_g = region_map[(g, 0, 1)][0]
                rows_g = 0
                for k in range(NBKT):
                    rows_g += region_map[(g, k, 1)][1]
                    if (g, k, 2) in region_map:
                        rows_g += region_map[(g, k, 2)][1]
                return r0_g, rows_g

            def _issue_calls(g, gt, itg, r0_g, ks):
                for cl in calls_by_g.get(g, []):
                    k, r0, nr = cl["k"], cl["row0"], cl["rows"]
                    if k not in ks:
                        continue
                    loc0 = (r0 - r0_g) // P      # tile offset in gt
                    nc.gpsimd.dma_gather(
                        out_ap=gt[:, loc0 * H:(loc0 + nr // P) * H].rearrange(
                            "p (t e) -> p t e", e=H),
                        in_ap=hp_fulls[k][:, :],
                        idxs_ap=itg[:, (r0 - r0_g) // 16:
                                    (r0 - r0_g + nr) // 16],
                        num_idxs=nr, num_idxs_reg=nr, elem_size=H,
                        single_packet=True, queue_num=cl["q"])

            # prefetch the next 2 groups' bucket-0 calls so the stream has
            # work while collectives 1-3 complete; reduces stay group-order
            PRE = min(2, cfg.NGRP)
            pre = {}
            for g in range(PRE):
                r0_g, rows_g = _grp_meta(g)
                gt = g1_pool.tile([P, (rows_g // P) * H], F32, tag="g1",
                                  name=f"g1_{g}")
                itg = idx_pool.tile([P, rows_g // 16], I16, tag="idxg",
                                    name=f"itg{g}")
                nc.sync.dma_start(
                    out=itg[:], in_=idx_d[:, r0_g // 16:(r0_g + rows_g) // 16])
                pre[g] = (gt, itg)
            for g in range(PRE):
                _issue_calls(g, pre[g][0], pre[g][1], _grp_meta(g)[0], {0})

            for g in range(cfg.NGRP):
                blocks = list(_group_blocks(cfg, g))
                gb = len(blocks)
                r0_g, rows_g = _grp_meta(g)
                if g in pre:
                    gt, itg = pre[g]
                    _issue_calls(g, gt, itg, r0_g, {1, 2, 3})
                else:
                    gt = g1_pool.tile([P, (rows_g // P) * H], F32, tag="g1",
                                      name=f"g1_{g}")
                    itg = idx_pool.tile([P, rows_g // 16], I16, tag="idxg",
                                        name=f"itg{g}")
                    nc.sync.dma_start(
                        out=itg[:],
                        in_=idx_d[:, r0_g // 16:(r0_g + rows_g) // 16])
                    _issue_calls(g, gt, itg, r0_g, {0, 1, 2, 3})

                # tier-1 reduce per (g, k), combine into zbuf slice
                acc = None
                for k in range(NBKT):
                    r0, nr = region_map[(g, k, 1)]
                    loc0 = (r0 - r0_g) // P
                    pt = part_pool.tile([P, gb * H], F32, tag="part",
                                        name=f"pt{g}_{k}")
                    # gt free layout: slot index s = loc0 + b*S_k + t
                    # reduce over t (innermost AP dim)
                    src_ap = bass.AP(
                        gt.tensor,
                        gt[:].offset + loc0 * H,
                        [gt[:].ap[0],
                         [S[k] * H, gb], [1, H], [H, S[k]]],
                    )
                    nc.vector.tensor_reduce(out=pt[:].rearrange(
                        "p (b e) -> p b e", e=H),
                        in_=src_ap, axis=AX.X, op=OP.add)
                    if acc is None:
                        acc = pt
                    else:
                        nc.vector.tensor_tensor(
                            out=acc[:], in0=acc[:], in1=pt[:], op=OP.add)
                zslice = zbuf[:, blocks[0] * H:(blocks[-1] + 1) * H]
                nc.scalar.activation(out=zslice, in_=acc[:], func=AF.Copy)

                # tier-2 pieces -> psum per block -> add into zbuf
                gp = pieces_by_g.get(g, [])
                by_blk = {}
                for (k, j, blk, ci) in gp:
                    by_blk.setdefault(blk, []).append((k, j, ci))
                OHMAX = 8
                for blk, plist in sorted(by_blk.items()):
                    # pieces of a block are a contiguous ci-run; build its
                    # one-hot matrices in batched stride-0-AP is_equal calls
                    npg = len(plist)
                    ci0 = plist[0][2]
                    assert [ci for (_, _, ci) in plist] == list(
                        range(ci0, ci0 + npg))
                    ps = psB.tile([P, H], F32, tag="ps2", name=f"ps2_{blk}")
                    for c0 in range(0, npg, OHMAX):
                        nch = min(OHMAX, npg - c0)
                        oh = st_pool.tile([P, nch * P], F32, tag="st",
                                          name=f"oh{g}_{blk}_{c0}")
                        in0 = bass.AP(dl2_sb.tensor,
                                      dl2_sb[:].offset + ci0 + c0,
                                      [dl2_sb[:].ap[0], [1, nch], [0, P]])
                        in1 = bass.AP(iota_f.tensor, iota_f[:].offset,
                                      [iota_f[:].ap[0], [0, nch], [1, P]])
                        nc.vector.tensor_tensor(out=oh[:], in0=in0, in1=in1,
                                                op=OP.is_equal)
                        for i in range(nch):
                            k, j, ci = plist[c0 + i]
                            r0, nr = region_map[(g, k, 2)]
                            loc0 = (r0 - r0_g) // P + j
                            nc.tensor.matmul(
                                out=ps[:], lhsT=oh[:, i * P:(i + 1) * P],
                                rhs=gt[:, loc0 * H:(loc0 + 1) * H],
                                start=(c0 + i == 0),
                                stop=(c0 + i == npg - 1))
                    nc.vector.tensor_tensor(
                        out=zbuf[:, blk * H:(blk + 1) * H],
                        in0=zbuf[:, blk * H:(blk + 1) * H],
                        in1=ps[:], op=OP.add)

                # ---- per-group epilogue (overlaps with next group) ----
                b0, b1 = blocks[0], blocks[-1] + 1
                gw = (b1 - b0) * H
                zs = e_pool.tile([P, gw], F32, tag="zs", name=f"zs{g}")
                nc.vector.tensor_tensor(out=zs[:],
                                        in0=zbuf[:, b0 * H:b1 * H],
                                        in1=hp_own[:, b0 * H:b1 * H],
                                        op=OP.add)
                dv = bass.AP(dinv_sb.tensor, dinv_sb[:].offset + b0,
                             [dinv_sb[:].ap[0], [1, b1 - b0], [0, H]])
                nc.vector.tensor_tensor(out=zs[:], in0=zs[:], in1=dv,
                                        op=OP.mult)
                if has_bias:
                    bb = bass.AP(b_sb.tensor, b_sb[:].offset,
                                 [b_sb[:].ap[0], [0, b1 - b0], [1, H]])
                    nc.vector.tensor_tensor(out=zs[:], in0=zs[:], in1=bb,
                                            op=OP.add)
                ta = e_pool.tile([P, gw], F32, tag="ta", name=f"ta{g}")
                nc.vector.tensor_scalar(out=ta[:], in0=zs[:], scalar1=0.0,
                                        scalar2=None, op0=OP.min)
                nc.scalar.activation(out=ta[:], in_=ta[:], func=AF.Exp)
                tb = e_pool.tile([P, gw], F32, tag="tb", name=f"tb{g}")
                nc.vector.tensor_scalar(out=tb[:], in0=zs[:], scalar1=0.0,
                                        scalar2=-1.0, op0=OP.max, op1=OP.add)
                nc.vector.tensor_tensor(out=zs[:], in0=ta[:], in1=tb[:],
                                        op=OP.add)
                so = bass.AP(out_d[:, :].tensor,
                             out_d[:, :].offset + b0 * P * H,
                             [[H, P], [P * H, b1 - b0], [1, H]])
                si = bass.AP(zs.tensor, zs[:].offset,
                             [zs[:].ap[0], [H, b1 - b0], [1, H]])
                nc.sync.dma_start(out=so, in_=si)
    nc.compile()
    return nc


def run(cfg, x, W, b, edge_index, **run_kwargs):
    in_maps, meta = _preprocess(cfg, x, W, b, edge_index)
    nc = _build_program(cfg, meta)
    res = bass_utils.run_bass_kernel_spmd(
        nc, in_maps, core_ids=list(range(cfg.C)), **run_kwargs)
    outs = [res.results[c]["out"][:cfg.NPC] for c in range(cfg.C)]
    full = np.concatenate(outs, axis=0).astype(np.float32)
    return full, res


def kernel(x, W, b, edge_index):
    cfg = Cfg(N=100000, E=1600000, F=128, H=64, C=8)
    out, _ = run(cfg, np.asarray(x), np.asarray(W), np.asarray(b),
                 np.asarray(edge_index))
    return out

